# revision 15
# baseline (speedup 1.0000x reference)
"""Trainium2 Bass kernel for nn_ClassConfusionLoss.

Self-contained: takes FULL inputs pred (64,64,128,128) f32, gt (64,64,128,128) i32,
shards the spatial W axis across 8 NeuronCores, computes per-core partial weighted
covariance M (64x64, as a 128x128 PSUM block pair), reduces on host and applies the
final row-normalization + trace (O(C^2), negligible).

Math: the reference's global scalars num_pos and S scale cov by alpha = num_pos/S,
which cancels in cov / cov.sum(axis=1). So only
M[c,k] = sum_p n_p*w_raw_p*x_pc*x_pk is needed, where x[b,c,w,h] =
pred[b,c,w,h]/(sum_c' pred[c,c',w,h] + eps)  (batch index c -- valid since B == C),
n = sum_c(gt==1), w_raw = 1 + exp(ent') with ent' = sum_c x*log(x+eps).

Layout per core (w-slab of 16, processed as 8 adjacent-w pairs):
  partition p = q*64 + c with q = b&1; free dims [t=b>>1 (32), j (2), h (128)].
  This makes each DMA descriptor span a contiguous (w,w+1)x(h) 256-element run
  in DRAM (1KB src / 512B dst), which keeps the cast DMA at full bus rate.

Per w-pair:
  pnx/gnx [128p, 32, 2, 128] bf16 <- 4 SWDGE cast DMAs (q halves x {pred, gt})
  D/N[h, j*64+b] via per-(q,t,j) matmuls vs ones (contraction over c)
  rp = 1/D; rt2[(q,c), (j,h)] = PE-transpose of rp (both q halves), bf16
  x = pnx * rt2-broadcast;  L = ln(x+eps);  xl = x*L
  E[h, j*64+b] via per-(q,t,j) matmuls of xl vs ones
  m = (exp(E)+1)*N;  sqm = exp(0.5*ln(m))      (single act table set: ln/exp/copy)
  per span of 8 b's: 16 PE transposes of x -> xt_ps[128,1024];
    z = xt_ps * sqm-broadcast  (z = sqrt(m) * x^T);  M_ps += z^T z per 128-col block
Host: M = sum_cores(M_ps[0:64,0:64] + M_ps[64:128,64:128]); cov /= cov.sum(1);
loss = (cov.sum() - trace)/C.
"""

import numpy as np

B, C, W, H = 64, 64, 128, 128
NCORES = 8
WS = W // NCORES          # 16 w's per core
NPAIR = WS // 2           # 8 w-pairs per core
EPS = 1e-12

# spans (of 64 total: wp*8+sp) whose z-scale op runs on gpsimd instead of DVE
Z_POOL_SPANS = frozenset()

_CACHE = {}


def _build_nc():
    from contextlib import ExitStack

    import concourse.bass as bass
    import concourse.tile as tile
    from concourse import bacc, masks, mybir
    from concourse.hw_specs import get_activation_tables

    F32 = mybir.dt.float32
    BF16 = mybir.dt.bfloat16
    I32 = mybir.dt.int32
    AF = mybir.ActivationFunctionType
    OP = mybir.AluOpType

    nc = bacc.Bacc("TRN2", target_bir_lowering=False, debug=False)

    pred_t = nc.dram_tensor("pred", [B, C, WS, H], F32, kind="ExternalInput")
    gt_t = nc.dram_tensor("gt", [B, C, WS, H], I32, kind="ExternalInput")
    mout_t = nc.dram_tensor("m_out", [128, 128], F32, kind="ExternalOutput")

    # DRAM strides (elements) of the shard tensor (B, C, WS, H)
    SB_, SC_, SW_, SH_ = C * WS * H, WS * H, H, 1

    with tile.TileContext(nc) as tc, ExitStack() as ctx:
        singles = ctx.enter_context(tc.tile_pool(name="singles", bufs=1))
        pred_pool = ctx.enter_context(tc.tile_pool(name="pred", bufs=2))
        gt_pool = ctx.enter_context(tc.tile_pool(name="gt", bufs=2))
        x_pool = ctx.enter_context(tc.tile_pool(name="x", bufs=2))
        l_pool = ctx.enter_context(tc.tile_pool(name="l", bufs=2))
        xl_pool = ctx.enter_context(tc.tile_pool(name="xl", bufs=2))
        sm_pool = ctx.enter_context(tc.tile_pool(name="sm", bufs=2))
        z_pool = ctx.enter_context(tc.tile_pool(name="z", bufs=4))
        ps_dn = ctx.enter_context(tc.tile_pool(name="ps_dn", bufs=2, space="PSUM"))
        ps_er = ctx.enter_context(tc.tile_pool(name="ps_er", bufs=2, space="PSUM"))
        ps_xt = ctx.enter_context(tc.tile_pool(name="ps_xt", bufs=3, space="PSUM"))
        ps_m = ctx.enter_context(tc.tile_pool(name="ps_m", bufs=1, space="PSUM"))

        ident_b = singles.tile([128, 128], BF16)
        masks.make_identity(nc, ident_b[:])
        ident_f = singles.tile([128, 128], F32)
        masks.make_identity(nc, ident_f[:])
        ones_c = singles.tile([128, 1], BF16)
        nc.vector.memset(ones_c[:], 1.0)
        eps_t = singles.tile([128, 1], F32)
        nc.vector.memset(eps_t[:], EPS)
        zero_t = singles.tile([128, 1], F32)
        nc.vector.memset(zero_t[:], 0.0)

        # Pin the ln+exp+copy activation table once so the compiler pass does
        # not insert a reload at every ln<->exp switch.
        tabs = get_activation_tables(nc.m.arch)
        set_id = next(
            i for i, s in enumerate(tabs.values())
            if AF.Ln in s and AF.Exp in s and AF.Copy in s
        )
        load_inst = mybir.InstLoadActFuncSet(
            name=nc.get_next_instruction_name(), act_func_set_id=set_id,
            ins=[], outs=[],
        )
        load_inst.engine = mybir.EngineType.Activation
        nc.scalar.add_instruction(load_inst)

        m_ps = ps_m.tile([128, 128], F32)
        first_mm = [True]
        st = {}  # per-wp live tiles for the software pipeline

        def emit_dma(wp):
            pnx = pred_pool.tile([128, 32, 2, 128], BF16)
            gnx = gt_pool.tile([128, 32, 2, 128], BF16)
            for q in range(2):
                off = wp * 2 * SW_ + q * SB_
                nc.gpsimd.dma_start(
                    out=pnx[q * 64:(q + 1) * 64],
                    in_=bass.AP(tensor=pred_t.ap().tensor, offset=off,
                                ap=[[SC_, 64], [2 * SB_, 32], [1, 256]]),
                )
                nc.gpsimd.dma_start(
                    out=gnx[q * 64:(q + 1) * 64],
                    in_=bass.AP(tensor=gt_t.ap().tensor, offset=off,
                                ap=[[SC_, 64], [2 * SB_, 32], [1, 256]]),
                )
            st[wp] = {"pnx": pnx, "gnx": gnx}

        def emit_head(wp):
            # D/N[h, j*64+b], rp = 1/D, rt2[(q,c),(j,h)], x, L
            s = st[wp]
            pnx, gnx = s["pnx"], s["gnx"]
            dn = ps_dn.tile([128, 256], F32)
            for q in range(2):
                on = ones_c[q * 64:(q + 1) * 64, :]
                for t in range(32):
                    b = 2 * t + q
                    for j in range(2):
                        col = j * 64 + b
                        nc.tensor.matmul(dn[:, col:col + 1],
                                         pnx[q * 64:(q + 1) * 64, t, j, :], on,
                                         start=True, stop=True,
                                         skip_group_check=True)
                        nc.tensor.matmul(dn[:, 128 + col:129 + col],
                                         gnx[q * 64:(q + 1) * 64, t, j, :], on,
                                         start=True, stop=True,
                                         skip_group_check=True)
            s["dn"] = dn

        def emit_rt(wp):
            # rp = 1/D, rt2[(q,c),(j,h)] = bf16(rp[h, j*64+c]) for both q
            s = st[wp]
            dn = s["dn"]
            rp = sm_pool.tile([128, 128], F32, tag="rp")
            nc.vector.reciprocal(rp[:], dn[:, 0:128])
            er = ps_er.tile([128, 384], F32)
            rt_ps = er[:, 128:384].rearrange("p (j h) -> p j h", j=2)
            for q in range(2):
                for j in range(2):
                    nc.tensor.matmul(rt_ps[q * 64:(q + 1) * 64, j],
                                     rp[:, j * 64:(j + 1) * 64], ident_f[:],
                                     is_transpose=True, start=True, stop=True,
                                     skip_group_check=True)
            rt2 = sm_pool.tile([128, 256], BF16, tag="rt2")
            nc.scalar.copy(rt2[:], er[:, 128:384])
            x = x_pool.tile([128, 32, 2, 128], BF16)
            L = l_pool.tile([128, 32, 2, 128], BF16)
            s.update(er=er, x=x, L=L, rt2=rt2)

        def emit_xL(wp, lo, hi):
            # one t-chunk of x = pnx*rt2-broadcast then L = ln(x+eps)
            s = st[wp]
            pnx, rt2, x, L = s["pnx"], s["rt2"], s["x"], s["L"]
            rt_b = bass.AP(tensor=rt2.tensor, offset=rt2.offset,
                           ap=[rt2.ap[0], [0, hi - lo], [128, 2], [1, 128]])
            nc.vector.tensor_mul(x[:, lo:hi], pnx[:, lo:hi], rt_b)
            nc.scalar.activation(L[:, lo:hi], x[:, lo:hi], AF.Ln,
                                 bias=eps_t[:], scale=1.0)

        def emit_xl(wp, lo, hi):
            s = st[wp]
            x, L = s["x"], s["L"]
            if "xl" not in s:
                s["xl"] = xl_pool.tile([128, 32, 2, 128], BF16, name="xl",
                                       tag="xl")
            nc.vector.tensor_mul(s["xl"][:, lo:hi], x[:, lo:hi], L[:, lo:hi])

        def emit_E(wp, lo, hi):
            # E[h, j*64+b] = sum_c xl for the given t-chunk
            s = st[wp]
            er, xl = s["er"], s["xl"]
            for q in range(2):
                on = ones_c[q * 64:(q + 1) * 64, :]
                for t in range(lo, hi):
                    b = 2 * t + q
                    for j in range(2):
                        col = j * 64 + b
                        nc.tensor.matmul(er[:, col:col + 1],
                                         xl[q * 64:(q + 1) * 64, t, j, :], on,
                                         start=True, stop=True,
                                         skip_group_check=True)

        def emit_expe(wp):
            s = st[wp]
            er = s["er"]
            expe = sm_pool.tile([128, 128], BF16, tag="expe")
            nc.scalar.activation(expe[:], er[:, 0:128], AF.Exp,
                                 bias=zero_t[:], scale=1.0)
            s["expe"] = expe

        def emit_sqm(wp):
            # sqm = sqrt((exp(E)+1)*N) = exp(0.5*ln(m))
            s = st[wp]
            dn, expe = s["dn"], s["expe"]
            m32 = sm_pool.tile([128, 128], F32, tag="m32")
            nc.vector.scalar_tensor_tensor(
                out=m32[:], in0=expe[:], scalar=1.0, in1=dn[:, 128:256],
                op0=OP.add, op1=OP.mult,
            )
            lnm = sm_pool.tile([128, 128], F32, tag="lnm")
            nc.scalar.activation(lnm[:], m32[:], AF.Ln, bias=eps_t[:], scale=1.0)
            sqm = sm_pool.tile([128, 128], BF16, tag="sqm")
            nc.scalar.activation(sqm[:], lnm[:], AF.Exp, bias=zero_t[:], scale=0.5)
            s["sqm"] = sqm

        def emit_tr(wp, sp):
            s = st[wp]
            x = s["x"]
            xt_ps = ps_xt.tile([128, 1024], BF16)
            for k in range(8):
                b = sp * 8 + k
                q, t = b & 1, b >> 1
                qs = slice(q * 64, (q + 1) * 64)
                for j in range(2):
                    nc.tensor.matmul(
                        xt_ps[:, k * 128 + j * 64:k * 128 + (j + 1) * 64],
                        x[qs, t, j, :], ident_b[qs, qs],
                        is_transpose=True, start=True, stop=True,
                        skip_group_check=True)
            s[("xt", sp)] = xt_ps

        def emit_z(wp, sp):
            s = st[wp]
            sqm = s["sqm"]
            xt_ps = s.pop(("xt", sp))
            z = z_pool.tile([128, 1024], BF16)
            sq_b = bass.AP(tensor=sqm.tensor, offset=sqm.offset + sp * 8,
                           ap=[sqm.ap[0], [1, 8], [64, 2], [0, 64]])
            eng = nc.gpsimd if (wp * 8 + sp) in Z_POOL_SPANS else nc.vector
            eng.tensor_mul(z[:], xt_ps[:], sq_b)
            s[("z", sp)] = z

        def emit_mains(wp, sp):
            z = st[wp].pop(("z", sp))
            for k in range(8):
                nc.tensor.matmul(
                    m_ps[:], z[:, k * 128:(k + 1) * 128],
                    z[:, k * 128:(k + 1) * 128],
                    start=first_mm[0],
                    stop=(wp == NPAIR - 1 and sp == 7 and k == 7),
                    skip_group_check=True,
                )
                first_mm[0] = False

        # Two-deep software pipeline: during wp's span phase we compute the
        # head (D/N, rp, rt2, x, L) for wp+2 and the tail (xl, E, sqm) for
        # wp+1.  Every cross-engine dependency then has >=1 full iteration of
        # slack, so the per-engine in-order streams never stall on the serial
        # D -> 1/D -> x -> ln -> x*ln -> E -> sqm chain.
        emit_dma(0)
        emit_dma(1)
        emit_dma(2)
        emit_head(0)
        emit_rt(0)
        for lo in (0, 16):
            emit_xL(0, lo, lo + 16)
            emit_xl(0, lo, lo + 16)
        emit_E(0, 0, 32)
        emit_expe(0)
        emit_sqm(0)
        emit_head(1)
        emit_rt(1)
        emit_xL(1, 0, 16)
        emit_xL(1, 16, 32)
        for wp in range(NPAIR):
            n1 = wp + 1 if wp + 1 < NPAIR else None
            n2 = wp + 2 if wp + 2 < NPAIR else None
            if wp + 3 < NPAIR:
                emit_dma(wp + 3)
            if n2 is not None:
                emit_head(n2)
            emit_tr(wp, 0)
            emit_tr(wp, 1)
            for sp in range(8):
                emit_z(wp, sp)
                if sp == 0 and n2 is not None:
                    emit_rt(n2)
                elif sp == 1 and n2 is not None:
                    emit_xL(n2, 0, 16)
                elif sp == 2 and n2 is not None:
                    emit_xL(n2, 16, 32)
                elif sp == 3 and n1 is not None:
                    emit_xl(n1, 0, 16)
                elif sp == 4 and n1 is not None:
                    emit_xl(n1, 16, 32)
                elif sp == 6 and n1 is not None:
                    emit_sqm(n1)
                emit_mains(wp, sp)
                if sp + 2 < 8:
                    emit_tr(wp, sp + 2)
                if sp == 4 and n1 is not None:
                    emit_E(n1, 0, 32)
                elif sp == 5 and n1 is not None:
                    emit_expe(n1)
            del st[wp]

        m_sb = singles.tile([128, 128], F32)
        nc.vector.tensor_copy(m_sb[:], m_ps[:])
        nc.sync.dma_start(out=mout_t.ap(), in_=m_sb[:])

    nc.compile()
    return nc


def _get_nc():
    if "nc" not in _CACHE:
        _CACHE["nc"] = _build_nc()
    return _CACHE["nc"]


def kernel(pred: np.ndarray, gt: np.ndarray) -> np.ndarray:
    from concourse.bass_utils import run_bass_kernel_spmd

    pred = np.ascontiguousarray(pred, dtype=np.float32)
    gt = np.ascontiguousarray(gt, dtype=np.int32)
    nc = _get_nc()

    in_maps = []
    for s in range(NCORES):
        in_maps.append({
            "pred": np.ascontiguousarray(pred[:, :, s * WS:(s + 1) * WS, :]),
            "gt": np.ascontiguousarray(gt[:, :, s * WS:(s + 1) * WS, :]),
        })
    res = run_bass_kernel_spmd(nc, in_maps, core_ids=list(range(NCORES)))

    M = np.zeros((64, 64), dtype=np.float32)
    for r in res.results:
        mo = r["m_out"]
        M += mo[0:64, 0:64] + mo[64:128, 64:128]
    cov = M / M.sum(axis=1)
    return np.float32((cov.sum() - np.trace(cov)) / C)


# revision 16
# speedup vs baseline: 1.0291x; 1.0291x over previous
"""Trainium2 Bass kernel for nn_ClassConfusionLoss.

Self-contained: takes FULL inputs pred (64,64,128,128) f32, gt (64,64,128,128) i32,
shards the spatial W axis across 8 NeuronCores, computes per-core partial weighted
covariance M (64x64, as a 128x128 PSUM block pair), reduces on host and applies the
final row-normalization + trace (O(C^2), negligible).

Math: the reference's global scalars num_pos and S scale cov by alpha = num_pos/S,
which cancels in cov / cov.sum(axis=1). So only
M[c,k] = sum_p n_p*w_raw_p*x_pc*x_pk is needed, where x[b,c,w,h] =
pred[b,c,w,h]/(sum_c' pred[c,c',w,h] + eps)  (batch index c -- valid since B == C),
n = sum_c(gt==1), w_raw = 1 + exp(ent') with ent' = sum_c x*log(x+eps).

Layout per core (w-slab of 16, processed as 8 adjacent-w pairs):
  partition p = q*64 + c with q = b&1; free dims [t=b>>1 (32), j (2), h (128)].
  This makes each DMA descriptor span a contiguous (w,w+1)x(h) 256-element run
  in DRAM (1KB src / 512B dst), which keeps the cast DMA at full bus rate.

Per w-pair:
  pnx/gnx [128p, 32, 2, 128] bf16 <- 4 SWDGE cast DMAs (q halves x {pred, gt})
  D/N[h, j*64+b] via per-(q,t,j) matmuls vs ones (contraction over c)
  rp = 1/D; rt2[(q,c), (j,h)] = PE-transpose of rp (both q halves), bf16
  x = pnx * rt2-broadcast;  L = ln(x+eps);  xl = x*L
  E[h, j*64+b] via per-(q,t,j) matmuls of xl vs ones
  m = (exp(E)+1)*N;  sqm = exp(0.5*ln(m))      (single act table set: ln/exp/copy)
  per span of 8 b's: 16 PE transposes of x -> xt_ps[128,1024];
    z = xt_ps * sqm-broadcast  (z = sqrt(m) * x^T);  M_ps += z^T z per 128-col block
Host: M = sum_cores(M_ps[0:64,0:64] + M_ps[64:128,64:128]); cov /= cov.sum(1);
loss = (cov.sum() - trace)/C.
"""

import numpy as np

B, C, W, H = 64, 64, 128, 128
NCORES = 8
WS = W // NCORES          # 16 w's per core
NPAIR = WS // 2           # 8 w-pairs per core
EPS = 1e-12

# spans (of 64 total: wp*8+sp) whose z-scale op runs on gpsimd instead of DVE
Z_POOL_SPANS = frozenset()

_CACHE = {}


def _build_nc():
    from contextlib import ExitStack

    import concourse.bass as bass
    import concourse.tile as tile
    from concourse import bacc, masks, mybir
    from concourse.hw_specs import get_activation_tables

    F32 = mybir.dt.float32
    BF16 = mybir.dt.bfloat16
    I32 = mybir.dt.int32
    AF = mybir.ActivationFunctionType
    OP = mybir.AluOpType

    nc = bacc.Bacc("TRN2", target_bir_lowering=False, debug=False)

    pred_t = nc.dram_tensor("pred", [B, C, WS, H], F32, kind="ExternalInput")
    gt_t = nc.dram_tensor("gt", [B, C, WS, H], I32, kind="ExternalInput")
    mout_t = nc.dram_tensor("m_out", [128, 128], F32, kind="ExternalOutput")

    # DRAM strides (elements) of the shard tensor (B, C, WS, H)
    SB_, SC_, SW_, SH_ = C * WS * H, WS * H, H, 1

    with tile.TileContext(nc) as tc, ExitStack() as ctx:
        singles = ctx.enter_context(tc.tile_pool(name="singles", bufs=1))
        pred_pool = ctx.enter_context(tc.tile_pool(name="pred", bufs=2))
        gt_pool = ctx.enter_context(tc.tile_pool(name="gt", bufs=2))
        x_pool = ctx.enter_context(tc.tile_pool(name="x", bufs=3))
        l_pool = ctx.enter_context(tc.tile_pool(name="l", bufs=2))
        xl_pool = ctx.enter_context(tc.tile_pool(name="xl", bufs=2))
        sm_pool = ctx.enter_context(tc.tile_pool(name="sm", bufs=2))
        z_pool = ctx.enter_context(tc.tile_pool(name="z", bufs=4))
        ps_dn = ctx.enter_context(tc.tile_pool(name="ps_dn", bufs=1, space="PSUM"))
        ps_er = ctx.enter_context(tc.tile_pool(name="ps_er", bufs=1, space="PSUM"))
        ps_xt = ctx.enter_context(tc.tile_pool(name="ps_xt", bufs=5, space="PSUM"))
        ps_m = ctx.enter_context(tc.tile_pool(name="ps_m", bufs=1, space="PSUM"))

        ident_b = singles.tile([128, 128], BF16)
        masks.make_identity(nc, ident_b[:])
        ident_f = singles.tile([128, 128], F32)
        masks.make_identity(nc, ident_f[:])
        ones_c = singles.tile([128, 1], BF16)
        nc.vector.memset(ones_c[:], 1.0)
        eps_t = singles.tile([128, 1], F32)
        nc.vector.memset(eps_t[:], EPS)
        zero_t = singles.tile([128, 1], F32)
        nc.vector.memset(zero_t[:], 0.0)

        # Pin the ln+exp+copy activation table once so the compiler pass does
        # not insert a reload at every ln<->exp switch.
        tabs = get_activation_tables(nc.m.arch)
        set_id = next(
            i for i, s in enumerate(tabs.values())
            if AF.Ln in s and AF.Exp in s and AF.Copy in s
        )
        load_inst = mybir.InstLoadActFuncSet(
            name=nc.get_next_instruction_name(), act_func_set_id=set_id,
            ins=[], outs=[],
        )
        load_inst.engine = mybir.EngineType.Activation
        nc.scalar.add_instruction(load_inst)

        m_ps = ps_m.tile([128, 128], F32)
        first_mm = [True]
        st = {}  # per-wp live tiles for the software pipeline

        def emit_dma(wp):
            pnx = pred_pool.tile([128, 32, 2, 128], BF16)
            gnx = gt_pool.tile([128, 32, 2, 128], BF16)
            for q in range(2):
                off = wp * 2 * SW_ + q * SB_
                nc.gpsimd.dma_start(
                    out=pnx[q * 64:(q + 1) * 64],
                    in_=bass.AP(tensor=pred_t.ap().tensor, offset=off,
                                ap=[[SC_, 64], [2 * SB_, 32], [1, 256]]),
                )
                nc.gpsimd.dma_start(
                    out=gnx[q * 64:(q + 1) * 64],
                    in_=bass.AP(tensor=gt_t.ap().tensor, offset=off,
                                ap=[[SC_, 64], [2 * SB_, 32], [1, 256]]),
                )
            st[wp] = {"pnx": pnx, "gnx": gnx}

        def emit_head(wp):
            # D/N[h, j*64+b], rp = 1/D, rt2[(q,c),(j,h)], x, L
            s = st[wp]
            pnx, gnx = s["pnx"], s["gnx"]
            dn = ps_dn.tile([128, 512], F32)
            for q in range(2):
                on = ones_c[q * 64:(q + 1) * 64, :]
                for t in range(32):
                    b = 2 * t + q
                    for j in range(2):
                        col = j * 64 + b
                        nc.tensor.matmul(dn[:, col:col + 1],
                                         pnx[q * 64:(q + 1) * 64, t, j, :], on,
                                         start=True, stop=True,
                                         skip_group_check=True)
                        nc.tensor.matmul(dn[:, 128 + col:129 + col],
                                         gnx[q * 64:(q + 1) * 64, t, j, :], on,
                                         start=True, stop=True,
                                         skip_group_check=True)
            s["dn"] = dn

        def emit_rt(wp):
            # rp = 1/D, rt2[(q,c),(j,h)] = bf16(rp[h, j*64+c]) for both q
            s = st[wp]
            dn = s["dn"]
            rp = sm_pool.tile([128, 128], F32, tag="rp")
            nc.vector.reciprocal(rp[:], dn[:, 0:128])
            nsb = sm_pool.tile([128, 128], BF16, tag="nsb")
            nc.scalar.copy(nsb[:], dn[:, 128:256])
            rt_ps = dn[:, 256:512].rearrange("p (j h) -> p j h", j=2)
            for q in range(2):
                for j in range(2):
                    nc.tensor.matmul(rt_ps[q * 64:(q + 1) * 64, j],
                                     rp[:, j * 64:(j + 1) * 64], ident_f[:],
                                     is_transpose=True, start=True, stop=True,
                                     skip_group_check=True)
            rt2 = sm_pool.tile([128, 256], BF16, tag="rt2")
            nc.scalar.copy(rt2[:], dn[:, 256:512])
            s["nsb"] = nsb
            x = x_pool.tile([128, 32, 2, 128], BF16)
            L = l_pool.tile([128, 32, 2, 128], BF16)
            s.update(x=x, L=L, rt2=rt2)

        def emit_xL(wp, lo, hi):
            # one t-chunk of x = pnx*rt2-broadcast then L = ln(x+eps)
            s = st[wp]
            pnx, rt2, x, L = s["pnx"], s["rt2"], s["x"], s["L"]
            rt_b = bass.AP(tensor=rt2.tensor, offset=rt2.offset,
                           ap=[rt2.ap[0], [0, hi - lo], [128, 2], [1, 128]])
            nc.vector.tensor_mul(x[:, lo:hi], pnx[:, lo:hi], rt_b)
            nc.scalar.activation(L[:, lo:hi], x[:, lo:hi], AF.Ln,
                                 bias=eps_t[:], scale=1.0)

        def emit_xl(wp, lo, hi):
            s = st[wp]
            x, L = s["x"], s["L"]
            if "xl" not in s:
                s["xl"] = xl_pool.tile([128, 32, 2, 128], BF16, name="xl",
                                       tag="xl")
            nc.vector.tensor_mul(s["xl"][:, lo:hi], x[:, lo:hi], L[:, lo:hi])

        def emit_E(wp, lo, hi):
            # E[h, j*64+b] = sum_c xl for the given t-chunk
            s = st[wp]
            xl = s["xl"]
            if "er" not in s:
                s["er"] = ps_er.tile([128, 128], F32, name="er", tag="er")
            er = s["er"]
            for q in range(2):
                on = ones_c[q * 64:(q + 1) * 64, :]
                for t in range(lo, hi):
                    b = 2 * t + q
                    for j in range(2):
                        col = j * 64 + b
                        nc.tensor.matmul(er[:, col:col + 1],
                                         xl[q * 64:(q + 1) * 64, t, j, :], on,
                                         start=True, stop=True,
                                         skip_group_check=True)

        def emit_expe(wp):
            s = st[wp]
            er = s["er"]
            expe = sm_pool.tile([128, 128], BF16, tag="expe")
            nc.scalar.activation(expe[:], er[:], AF.Exp,
                                 bias=zero_t[:], scale=1.0)
            s["expe"] = expe

        def emit_sqm(wp):
            # sqm = sqrt((exp(E)+1)*N) = exp(0.5*ln(m))
            s = st[wp]
            nsb, expe = s["nsb"], s["expe"]
            m32 = sm_pool.tile([128, 128], F32, tag="m32")
            nc.vector.scalar_tensor_tensor(
                out=m32[:], in0=expe[:], scalar=1.0, in1=nsb[:],
                op0=OP.add, op1=OP.mult,
            )
            lnm = sm_pool.tile([128, 128], F32, tag="lnm")
            nc.scalar.activation(lnm[:], m32[:], AF.Ln, bias=eps_t[:], scale=1.0)
            sqm = sm_pool.tile([128, 128], BF16, tag="sqm")
            nc.scalar.activation(sqm[:], lnm[:], AF.Exp, bias=zero_t[:], scale=0.5)
            s["sqm"] = sqm

        def emit_tr(wp, sp):
            s = st[wp]
            x = s["x"]
            xt_ps = ps_xt.tile([128, 1024], BF16)
            for k in range(8):
                b = sp * 8 + k
                q, t = b & 1, b >> 1
                qs = slice(q * 64, (q + 1) * 64)
                for j in range(2):
                    nc.tensor.matmul(
                        xt_ps[:, k * 128 + j * 64:k * 128 + (j + 1) * 64],
                        x[qs, t, j, :], ident_b[qs, qs],
                        is_transpose=True, start=True, stop=True,
                        skip_group_check=True)
            s[("xt", sp)] = xt_ps

        def emit_z(wp, sp):
            s = st[wp]
            sqm = s["sqm"]
            xt_ps = s.pop(("xt", sp))
            z = z_pool.tile([128, 1024], BF16)
            sq_b = bass.AP(tensor=sqm.tensor, offset=sqm.offset + sp * 8,
                           ap=[sqm.ap[0], [1, 8], [64, 2], [0, 64]])
            eng = nc.gpsimd if (wp * 8 + sp) in Z_POOL_SPANS else nc.vector
            eng.tensor_mul(z[:], xt_ps[:], sq_b)
            s[("z", sp)] = z

        def emit_mains(wp, sp):
            z = st[wp].pop(("z", sp))
            for k in range(8):
                nc.tensor.matmul(
                    m_ps[:], z[:, k * 128:(k + 1) * 128],
                    z[:, k * 128:(k + 1) * 128],
                    start=first_mm[0],
                    stop=(wp == NPAIR - 1 and sp == 7 and k == 7),
                    skip_group_check=True,
                )
                first_mm[0] = False

        # Two-deep software pipeline: during wp's span phase we compute the
        # head (D/N, rp, rt2, x, L) for wp+2 and the tail (xl, E, sqm) for
        # wp+1.  Every cross-engine dependency then has >=1 full iteration of
        # slack, so the per-engine in-order streams never stall on the serial
        # D -> 1/D -> x -> ln -> x*ln -> E -> sqm chain.
        emit_dma(0)
        emit_dma(1)
        emit_dma(2)
        emit_head(0)
        emit_rt(0)
        for lo in (0, 16):
            emit_xL(0, lo, lo + 16)
            emit_xl(0, lo, lo + 16)
        emit_E(0, 0, 32)
        emit_expe(0)
        emit_sqm(0)
        emit_head(1)
        emit_rt(1)
        emit_xL(1, 0, 16)
        emit_xL(1, 16, 32)
        for wp in range(NPAIR):
            n1 = wp + 1 if wp + 1 < NPAIR else None
            n2 = wp + 2 if wp + 2 < NPAIR else None
            if wp + 3 < NPAIR:
                emit_dma(wp + 3)
            if n2 is not None:
                emit_head(n2)
            emit_tr(wp, 0)
            emit_tr(wp, 1)
            for sp in range(8):
                emit_z(wp, sp)
                if sp == 0 and n2 is not None:
                    emit_rt(n2)
                elif sp == 1 and n2 is not None:
                    emit_xL(n2, 0, 16)
                elif sp == 2 and n2 is not None:
                    emit_xL(n2, 16, 32)
                elif sp == 3 and n1 is not None:
                    emit_xl(n1, 0, 16)
                elif sp == 4 and n1 is not None:
                    emit_xl(n1, 16, 32)
                elif sp == 6 and n1 is not None:
                    emit_sqm(n1)
                emit_mains(wp, sp)
                if sp + 2 < 8:
                    emit_tr(wp, sp + 2)
                if sp == 4 and n1 is not None:
                    emit_E(n1, 0, 32)
                elif sp == 5 and n1 is not None:
                    emit_expe(n1)
            del st[wp]

        m_sb = singles.tile([128, 128], F32)
        nc.vector.tensor_copy(m_sb[:], m_ps[:])
        nc.sync.dma_start(out=mout_t.ap(), in_=m_sb[:])

    nc.compile()
    return nc


def _get_nc():
    if "nc" not in _CACHE:
        _CACHE["nc"] = _build_nc()
    return _CACHE["nc"]


def kernel(pred: np.ndarray, gt: np.ndarray) -> np.ndarray:
    from concourse.bass_utils import run_bass_kernel_spmd

    pred = np.ascontiguousarray(pred, dtype=np.float32)
    gt = np.ascontiguousarray(gt, dtype=np.int32)
    nc = _get_nc()

    in_maps = []
    for s in range(NCORES):
        in_maps.append({
            "pred": np.ascontiguousarray(pred[:, :, s * WS:(s + 1) * WS, :]),
            "gt": np.ascontiguousarray(gt[:, :, s * WS:(s + 1) * WS, :]),
        })
    res = run_bass_kernel_spmd(nc, in_maps, core_ids=list(range(NCORES)))

    M = np.zeros((64, 64), dtype=np.float32)
    for r in res.results:
        mo = r["m_out"]
        M += mo[0:64, 0:64] + mo[64:128, 64:128]
    cov = M / M.sum(axis=1)
    return np.float32((cov.sum() - np.trace(cov)) / C)


# revision 19
# speedup vs baseline: 1.0425x; 1.0130x over previous
"""Trainium2 Bass kernel for nn_ClassConfusionLoss.

Self-contained: takes FULL inputs pred (64,64,128,128) f32, gt (64,64,128,128) i32,
shards the spatial W axis across 8 NeuronCores, computes per-core partial weighted
covariance M (64x64, as a 128x128 PSUM block pair), reduces on host and applies the
final row-normalization + trace (O(C^2), negligible).

Math: the reference's global scalars num_pos and S scale cov by alpha = num_pos/S,
which cancels in cov / cov.sum(axis=1). So only
M[c,k] = sum_p n_p*w_raw_p*x_pc*x_pk is needed, where x[b,c,w,h] =
pred[b,c,w,h]/(sum_c' pred[c,c',w,h] + eps)  (batch index c -- valid since B == C),
n = sum_c(gt==1), w_raw = 1 + exp(ent') with ent' = sum_c x*log(x+eps).

Layout per core (w-slab of 16, processed as 8 adjacent-w pairs):
  partition p = q*64 + c with q = b&1; free dims [t=b>>1 (32), j (2), h (128)].
  This makes each DMA descriptor span a contiguous (w,w+1)x(h) 256-element run
  in DRAM (1KB src / 512B dst), which keeps the cast DMA at full bus rate.

Per w-pair:
  pnx/gnx [128p, 32, 2, 128] bf16 <- 4 SWDGE cast DMAs (q halves x {pred, gt})
  D/N[h, j*64+b] via per-(q,t,j) matmuls vs ones (contraction over c)
  rp = 1/D; rt2[(q,c), (j,h)] = PE-transpose of rp (both q halves), bf16
  x = pnx * rt2-broadcast;  L = ln(x+eps);  xl = x*L
  E[h, j*64+b] via per-(q,t,j) matmuls of xl vs ones
  m = (exp(E)+1)*N;  sqm = exp(0.5*ln(m))      (single act table set: ln/exp/copy)
  per span of 8 b's: 16 PE transposes of x -> xt_ps[128,1024];
    z = xt_ps * sqm-broadcast  (z = sqrt(m) * x^T);  M_ps += z^T z per 128-col block
Host: M = sum_cores(M_ps[0:64,0:64] + M_ps[64:128,64:128]); cov /= cov.sum(1);
loss = (cov.sum() - trace)/C.
"""

import numpy as np

B, C, W, H = 64, 64, 128, 128
NCORES = 8
WS = W // NCORES          # 16 w's per core
NPAIR = WS // 2           # 8 w-pairs per core
EPS = 1e-12

# spans (of 64 total: wp*8+sp) whose z-scale op runs on gpsimd instead of DVE
Z_POOL_SPANS = frozenset()

_CACHE = {}


def _build_nc():
    from contextlib import ExitStack

    import concourse.bass as bass
    import concourse.tile as tile
    from concourse import bacc, masks, mybir
    from concourse.hw_specs import get_activation_tables

    F32 = mybir.dt.float32
    BF16 = mybir.dt.bfloat16
    I32 = mybir.dt.int32
    AF = mybir.ActivationFunctionType
    OP = mybir.AluOpType

    nc = bacc.Bacc("TRN2", target_bir_lowering=False, debug=False)

    pred_t = nc.dram_tensor("pred", [B, C, WS, H], F32, kind="ExternalInput")
    gt_t = nc.dram_tensor("gt", [B, C, WS, H], I32, kind="ExternalInput")
    mout_t = nc.dram_tensor("m_out", [128, 128], F32, kind="ExternalOutput")

    # DRAM strides (elements) of the shard tensor (B, C, WS, H)
    SB_, SC_, SW_, SH_ = C * WS * H, WS * H, H, 1

    with tile.TileContext(nc) as tc, ExitStack() as ctx:
        singles = ctx.enter_context(tc.tile_pool(name="singles", bufs=1))
        pred_pool = ctx.enter_context(tc.tile_pool(name="pred", bufs=2))
        gt_pool = ctx.enter_context(tc.tile_pool(name="gt", bufs=2))
        x_pool = ctx.enter_context(tc.tile_pool(name="x", bufs=3))
        l_pool = ctx.enter_context(tc.tile_pool(name="l", bufs=2))
        xl_pool = ctx.enter_context(tc.tile_pool(name="xl", bufs=2))
        sm_pool = ctx.enter_context(tc.tile_pool(name="sm", bufs=2))
        z_pool = ctx.enter_context(tc.tile_pool(name="z", bufs=4))
        ps_dn = ctx.enter_context(tc.tile_pool(name="ps_dn", bufs=1, space="PSUM"))
        ps_er = ctx.enter_context(tc.tile_pool(name="ps_er", bufs=2, space="PSUM"))
        ps_xt = ctx.enter_context(tc.tile_pool(name="ps_xt", bufs=4, space="PSUM"))
        ps_m = ctx.enter_context(tc.tile_pool(name="ps_m", bufs=1, space="PSUM"))

        ident_b = singles.tile([128, 128], BF16)
        masks.make_identity(nc, ident_b[:])
        ident_f = singles.tile([128, 128], F32)
        masks.make_identity(nc, ident_f[:])
        ones_c = singles.tile([128, 1], BF16)
        nc.vector.memset(ones_c[:], 1.0)
        eps_t = singles.tile([128, 1], F32)
        nc.vector.memset(eps_t[:], EPS)
        zero_t = singles.tile([128, 1], F32)
        nc.vector.memset(zero_t[:], 0.0)

        # Pin the ln+exp+copy activation table once so the compiler pass does
        # not insert a reload at every ln<->exp switch.
        tabs = get_activation_tables(nc.m.arch)
        set_id = next(
            i for i, s in enumerate(tabs.values())
            if AF.Ln in s and AF.Exp in s and AF.Copy in s
        )
        load_inst = mybir.InstLoadActFuncSet(
            name=nc.get_next_instruction_name(), act_func_set_id=set_id,
            ins=[], outs=[],
        )
        load_inst.engine = mybir.EngineType.Activation
        nc.scalar.add_instruction(load_inst)

        m_ps = ps_m.tile([128, 128], F32)
        first_mm = [True]
        st = {}  # per-wp live tiles for the software pipeline

        def emit_dma(wp):
            pnx = pred_pool.tile([128, 32, 2, 128], BF16)
            gnx = gt_pool.tile([128, 32, 2, 128], BF16)
            for q in range(2):
                off = wp * 2 * SW_ + q * SB_
                nc.gpsimd.dma_start(
                    out=pnx[q * 64:(q + 1) * 64],
                    in_=bass.AP(tensor=pred_t.ap().tensor, offset=off,
                                ap=[[SC_, 64], [2 * SB_, 32], [1, 256]]),
                )
                nc.gpsimd.dma_start(
                    out=gnx[q * 64:(q + 1) * 64],
                    in_=bass.AP(tensor=gt_t.ap().tensor, offset=off,
                                ap=[[SC_, 64], [2 * SB_, 32], [1, 256]]),
                )
            st[wp] = {"pnx": pnx, "gnx": gnx}

        def emit_head(wp):
            # D/N[h, j*64+b], rp = 1/D, rt2[(q,c),(j,h)], x, L
            s = st[wp]
            pnx, gnx = s["pnx"], s["gnx"]
            dn = ps_dn.tile([128, 256], F32)
            for q in range(2):
                on = ones_c[q * 64:(q + 1) * 64, :]
                for t in range(32):
                    b = 2 * t + q
                    for j in range(2):
                        col = j * 64 + b
                        nc.tensor.matmul(dn[:, col:col + 1],
                                         pnx[q * 64:(q + 1) * 64, t, j, :], on,
                                         start=True, stop=True,
                                         skip_group_check=True)
                        nc.tensor.matmul(dn[:, 128 + col:129 + col],
                                         gnx[q * 64:(q + 1) * 64, t, j, :], on,
                                         start=True, stop=True,
                                         skip_group_check=True)
            s["dn"] = dn

        def emit_rt(wp):
            # rp = 1/D, rt2[(q,c),(j,h)] = bf16(rp[h, j*64+c]) for both q
            s = st[wp]
            dn = s["dn"]
            rp = sm_pool.tile([128, 128], F32, tag="rp")
            nc.vector.reciprocal(rp[:], dn[:, 0:128])
            nsb = sm_pool.tile([128, 128], BF16, tag="nsb")
            nc.scalar.copy(nsb[:], dn[:, 128:256])
            er = ps_er.tile([128, 384], F32)
            s["er"] = er
            rt_ps = er[:, 128:384].rearrange("p (j h) -> p j h", j=2)
            for q in range(2):
                for j in range(2):
                    nc.tensor.matmul(rt_ps[q * 64:(q + 1) * 64, j],
                                     rp[:, j * 64:(j + 1) * 64], ident_f[:],
                                     is_transpose=True, start=True, stop=True,
                                     skip_group_check=True)
            rt2 = sm_pool.tile([128, 256], BF16, tag="rt2")
            nc.scalar.copy(rt2[:], er[:, 128:384])
            s["nsb"] = nsb
            x = x_pool.tile([128, 32, 2, 128], BF16)
            L = l_pool.tile([128, 32, 2, 128], BF16)
            s.update(x=x, L=L, rt2=rt2)

        def emit_xL(wp, lo, hi):
            # one t-chunk of x = pnx*rt2-broadcast then L = ln(x+eps)
            s = st[wp]
            pnx, rt2, x, L = s["pnx"], s["rt2"], s["x"], s["L"]
            rt_b = bass.AP(tensor=rt2.tensor, offset=rt2.offset,
                           ap=[rt2.ap[0], [0, hi - lo], [128, 2], [1, 128]])
            nc.vector.tensor_mul(x[:, lo:hi], pnx[:, lo:hi], rt_b)
            nc.scalar.activation(L[:, lo:hi], x[:, lo:hi], AF.Ln,
                                 bias=eps_t[:], scale=1.0)

        def emit_xl(wp, lo, hi):
            s = st[wp]
            x, L = s["x"], s["L"]
            if "xl" not in s:
                s["xl"] = xl_pool.tile([128, 32, 2, 128], BF16, name="xl",
                                       tag="xl")
            nc.vector.tensor_mul(s["xl"][:, lo:hi], x[:, lo:hi], L[:, lo:hi])

        def emit_E(wp, lo, hi):
            # E[h, j*64+b] = sum_c xl for the given t-chunk
            s = st[wp]
            xl = s["xl"]
            er = s["er"]
            for q in range(2):
                on = ones_c[q * 64:(q + 1) * 64, :]
                for t in range(lo, hi):
                    b = 2 * t + q
                    for j in range(2):
                        col = j * 64 + b
                        nc.tensor.matmul(er[:, col:col + 1],
                                         xl[q * 64:(q + 1) * 64, t, j, :], on,
                                         start=True, stop=True,
                                         skip_group_check=True)

        def emit_expe(wp):
            s = st[wp]
            er = s["er"]
            expe = sm_pool.tile([128, 128], BF16, tag="expe")
            nc.scalar.activation(expe[:], er[:, 0:128], AF.Exp,
                                 bias=zero_t[:], scale=1.0)
            s["expe"] = expe

        def emit_sqm(wp):
            # sqm = sqrt((exp(E)+1)*N) = exp(0.5*ln(m))
            s = st[wp]
            nsb, expe = s["nsb"], s["expe"]
            m32 = sm_pool.tile([128, 128], F32, tag="m32")
            nc.vector.scalar_tensor_tensor(
                out=m32[:], in0=expe[:], scalar=1.0, in1=nsb[:],
                op0=OP.add, op1=OP.mult,
            )
            lnm = sm_pool.tile([128, 128], F32, tag="lnm")
            nc.scalar.activation(lnm[:], m32[:], AF.Ln, bias=eps_t[:], scale=1.0)
            sqm = sm_pool.tile([128, 128], BF16, tag="sqm")
            nc.scalar.activation(sqm[:], lnm[:], AF.Exp, bias=zero_t[:], scale=0.5)
            s["sqm"] = sqm

        def emit_tr(wp, sp):
            s = st[wp]
            x = s["x"]
            xt_ps = ps_xt.tile([128, 1024], BF16)
            for k in range(8):
                b = sp * 8 + k
                q, t = b & 1, b >> 1
                qs = slice(q * 64, (q + 1) * 64)
                for j in range(2):
                    nc.tensor.matmul(
                        xt_ps[:, k * 128 + j * 64:k * 128 + (j + 1) * 64],
                        x[qs, t, j, :], ident_b[qs, qs],
                        is_transpose=True, start=True, stop=True,
                        skip_group_check=True)
            s[("xt", sp)] = xt_ps

        def emit_z(wp, sp):
            s = st[wp]
            sqm = s["sqm"]
            xt_ps = s.pop(("xt", sp))
            z = z_pool.tile([128, 1024], BF16)
            sq_b = bass.AP(tensor=sqm.tensor, offset=sqm.offset + sp * 8,
                           ap=[sqm.ap[0], [1, 8], [64, 2], [0, 64]])
            eng = nc.gpsimd if (wp * 8 + sp) in Z_POOL_SPANS else nc.vector
            eng.tensor_mul(z[:], xt_ps[:], sq_b)
            s[("z", sp)] = z

        def emit_mains(wp, sp):
            z = st[wp].pop(("z", sp))
            for k in range(8):
                nc.tensor.matmul(
                    m_ps[:], z[:, k * 128:(k + 1) * 128],
                    z[:, k * 128:(k + 1) * 128],
                    start=first_mm[0],
                    stop=(wp == NPAIR - 1 and sp == 7 and k == 7),
                    skip_group_check=True,
                )
                first_mm[0] = False

        # Two-deep software pipeline: during wp's span phase we compute the
        # head (D/N, rp, rt2, x, L) for wp+2 and the tail (xl, E, sqm) for
        # wp+1.  Every cross-engine dependency then has >=1 full iteration of
        # slack, so the per-engine in-order streams never stall on the serial
        # D -> 1/D -> x -> ln -> x*ln -> E -> sqm chain.
        emit_dma(0)
        emit_dma(1)
        emit_dma(2)
        emit_head(0)
        emit_rt(0)
        for lo in (0, 16):
            emit_xL(0, lo, lo + 16)
            emit_xl(0, lo, lo + 16)
        emit_E(0, 0, 32)
        emit_expe(0)
        emit_sqm(0)
        emit_head(1)
        emit_rt(1)
        emit_xL(1, 0, 16)
        emit_xL(1, 16, 32)
        emit_tr(0, 0)
        emit_tr(0, 1)
        for wp in range(NPAIR):
            n1 = wp + 1 if wp + 1 < NPAIR else None
            n2 = wp + 2 if wp + 2 < NPAIR else None
            if wp + 3 < NPAIR:
                emit_dma(wp + 3)
            if n2 is not None:
                emit_head(n2)
            for sp in range(8):
                emit_z(wp, sp)
                if sp == 0 and n2 is not None:
                    emit_rt(n2)
                elif sp == 1 and n2 is not None:
                    emit_xL(n2, 0, 16)
                elif sp == 2 and n2 is not None:
                    emit_xL(n2, 16, 32)
                elif sp == 3 and n1 is not None:
                    emit_xl(n1, 0, 16)
                elif sp == 4 and n1 is not None:
                    emit_xl(n1, 16, 32)
                elif sp == 6 and n1 is not None:
                    emit_sqm(n1)
                emit_mains(wp, sp)
                if sp + 2 < 8:
                    emit_tr(wp, sp + 2)
                elif n1 is not None:
                    emit_tr(n1, sp - 6)
                if sp == 4 and n1 is not None:
                    emit_E(n1, 0, 32)
                elif sp == 5 and n1 is not None:
                    emit_expe(n1)
            del st[wp]

        m_sb = singles.tile([128, 128], F32)
        nc.vector.tensor_copy(m_sb[:], m_ps[:])
        nc.sync.dma_start(out=mout_t.ap(), in_=m_sb[:])

    nc.compile()
    return nc


def _get_nc():
    if "nc" not in _CACHE:
        _CACHE["nc"] = _build_nc()
    return _CACHE["nc"]


def kernel(pred: np.ndarray, gt: np.ndarray) -> np.ndarray:
    from concourse.bass_utils import run_bass_kernel_spmd

    pred = np.ascontiguousarray(pred, dtype=np.float32)
    gt = np.ascontiguousarray(gt, dtype=np.int32)
    nc = _get_nc()

    in_maps = []
    for s in range(NCORES):
        in_maps.append({
            "pred": np.ascontiguousarray(pred[:, :, s * WS:(s + 1) * WS, :]),
            "gt": np.ascontiguousarray(gt[:, :, s * WS:(s + 1) * WS, :]),
        })
    res = run_bass_kernel_spmd(nc, in_maps, core_ids=list(range(NCORES)))

    M = np.zeros((64, 64), dtype=np.float32)
    for r in res.results:
        mo = r["m_out"]
        M += mo[0:64, 0:64] + mo[64:128, 64:128]
    cov = M / M.sum(axis=1)
    return np.float32((cov.sum() - np.trace(cov)) / C)
